# revision 6
# baseline (speedup 1.0000x reference)
"""SRP layer distributed Bass kernel for TRN2 (v13).

Math (full problem): out = Psi_c @ x.T @ x with Psi_c = Psi - rowmean(Psi).
  x [D, N] f32, Psi [O, N] f32, out [O, N] f32  (D=4096, N=8192, O=2048)

Distribution over 8 cores as a 4x2 grid: core c -> (i = c % 4: n-quarter,
j = c // 4: o-half). The host pre-centers Psi (global row-mean), pre-slices,
pre-transposes, and pre-casts to bf16, so the device does NOTHING but the
two GEMMs and the tmp AllReduce:

Per core (NL = N/4 = 2048, OL = O/2 = 1024):
  xT   [NL, D]  bf16  (x_i.T)        - mm1 stationary operand
  x    [D, NL]  bf16  (x_i)          - mm2 moving operand
  psiT [NL, OL] bf16  (Psi_c_ji.T)   - mm1 moving operand
  out  [OL, NL] f32

mm1: tmpT[d, o] = sum_n xT[n, d] * psiT[n, o]   (partial over local n)
     -> bf16 -> DRAM in 5 d-chunks (4,4,8,8,8 d-tiles), each AllReduce'd
     over the 4 cores of the same o-half as soon as it is ready (the small
     leading chunks launch the first collective early, absorbing the ~30us
     cross-core start skew; the chain overlaps mm1 + mm2 pass A).
mm2: out[o, n] = sum_d tmpT[d, o] * x[d, n], two kd-half passes so pass A
     (kd 0..15, AR chunks 0-2) runs while chunks 3-4 still AllReduce;
     pass B adds and streams the f32 result out.

Perf notes (hardware-measured):
- Sustained MM pitch is 263 ns (512 cyc at 1.95 GHz: the package SW
  throttler holds k=13/16 under full-PE load; 2.4 GHz only in bursts),
  so the PE floor here is 2048 x 263 = 539 us.
- v12 measured 601 us with a 32 us PE bubble at the mm1->mm2 switch:
  the sb1 pool-close barrier counts ALL prior DMA-queue completions, and
  the tmp_sb loads for AR chunks 3-4 (issued inside the sb1 scope) kept
  the barrier - and with it the vector queue, the first out_part copies,
  and AR4's trigger - hostage to AllReduce-3 (~317 us > mm1 end ~298 us).
  v13 issues every AR- or mm2-dependent load AFTER the sb1 close, so the
  barrier clears the moment mm1's own traffic drains (~298 us).
- Bulk loads are single multi-dim DMAs (one per xT chunk / psiT half
  piece / tmp chunk / x2b block) spread over scalar+gpsimd+vector+sync
  queues: a single queue sustains only ~190 GB/s, and dt0 needs 5 MiB
  in its first ~19 us, so the initial fill uses four queues in parallel.
- Matmul operands slice a few BIG consolidated SBUF tiles: per-matmul
  semaphore waits otherwise break LDWEIGHTS pipelining (+45 ns/MM).
- gpsimd carries the collectives (they block their issuing queue) plus
  one initial psiT piece; nothing time-critical queues behind an AR.
- t1s staging is 24 deep and mm1 holds 6 PSUM banks so mm1 coasts
  through the DMA blackout during each AllReduce's transfer phase.
"""

from contextlib import ExitStack

import concourse.bacc as bacc
import concourse.mybir as mybir
import concourse.tile as tile

F32 = mybir.dt.float32
BF = mybir.dt.bfloat16


def build_srp_kernel(
    D=4096,
    NL=2048,
    OL=1024,
    n_cores=8,
    groups=((0, 1, 2, 3), (4, 5, 6, 7)),
):
    DT = D // 128    # 32 d-tiles (tmpT partition tiles / mm2 contraction)
    NT = NL // 128   # 16 n-tiles (mm1 contraction)
    OC = OL // 512   # 2  o-chunks (mm1 free cols)
    NCH = NL // 512  # 4  n-chunks (mm2 free cols)
    OT = OL // 128   # 8  o-tiles (mm2 output partition tiles)
    DC = D // 512    # 8  xT d-chunks (streamed)
    # AllReduce chunk sizes in d-tiles: small leading chunks so the first
    # collective launches early (it absorbs the ~30us cross-core arrival
    # skew) and the chain covering pass A (dt 0..15) completes well before
    # mm1 ends; chunks 0..2 -> mm2 pass A, chunks 3..4 -> pass B.
    CH_DT = (4, 4, 8, 8, 8)
    CH_START = [sum(CH_DT[:i]) for i in range(len(CH_DT))]
    KH = DT // 2     # 16 kd per mm2 pass

    groups = [list(g) for g in groups]

    nc = bacc.Bacc("TRN2", target_bir_lowering=False, debug=False,
                   num_devices=n_cores)
    xT_ext = nc.dram_tensor("xT", [NL, D], BF, kind="ExternalInput")
    x_ext = nc.dram_tensor("x", [D, NL], BF, kind="ExternalInput")
    psiT_ext = nc.dram_tensor("psiT", [NL, OL], BF, kind="ExternalInput")
    out_ext = nc.dram_tensor("out", [OL, NL], F32, kind="ExternalOutput")

    # [p, nt, c] views: partition-first on both sides of every bulk DMA.
    psiT_r = psiT_ext.rearrange("(nt p) (oc c) -> oc p nt c", p=128, c=512)
    xT_r = xT_ext.rearrange("(nt p) (dc c) -> dc p nt c", p=128, c=512)
    x_r = x_ext.rearrange("(ph dq p) (ncn c) -> ph ncn p dq c",
                          dq=KH, p=128, c=512)

    with ExitStack() as stack:
        tc = stack.enter_context(tile.TileContext(nc))
        dram = stack.enter_context(tc.tile_pool(name="dram", bufs=1, space="DRAM"))
        ps = stack.enter_context(tc.tile_pool(name="ps", bufs=1, space="PSUM"))
        sbl = stack.enter_context(tc.tile_pool(name="sbl", bufs=1))

        tmp_in = [dram.tile([CH_DT[q] * 128, OL], BF, tag=f"tmp_in{q}", bufs=1,
                            name=f"tmp_in{q}") for q in range(len(CH_DT))]
        tmp_out = [dram.tile([CH_DT[q] * 128, OL], BF, tag=f"tmp_out{q}", bufs=1,
                             name=f"tmp_out{q}") for q in range(len(CH_DT))]

        tmp_sb = [sbl.tile([128, KH * OL], BF, tag="tmp_sb", bufs=2,
                           name=f"tmp_sb{p}") for p in range(2)]
        x2b = {}

        # ============ mm1 ============
        with tc.tile_pool(name="sb1", bufs=1) as sb1:
            # psiT split into two o-halves, each one big tile [128, NT*512]
            # (block nt at cols nt*512..). Four 4-nt pieces per half so the
            # first MMs start as soon as piece 0 lands; oc0 on scalar,
            # oc1 on gpsimd (idle until AR0, ~70us in).
            psiT_sb = [sb1.tile([128, NT * 512], BF, tag=f"psiT{oc}", bufs=1,
                                name=f"psiT{oc}") for oc in range(OC)]
            for oc in range(OC):
                eng = nc.scalar if oc == 0 else nc.gpsimd
                dst3 = psiT_sb[oc][:, :].rearrange("p (nt c) -> p nt c", c=512)
                for g in range(4):
                    eng.dma_start(dst3[:, 4 * g:4 * (g + 1)],
                                  psiT_r[oc, :, 4 * g:4 * (g + 1)])

            # xT chunks: one big tile per 512-d-col chunk [128, NT*512],
            # rotating through 3 buffers, all on sync (DVE cannot issue
            # DMAs); chunk 0 in 4 pieces so MM0 starts at piece 0.
            xtc = {}

            def load_chunk(dc, eng, pieces=1):
                t = sb1.tile([128, NT * 512], BF, tag="xTc", bufs=3,
                             name=f"xTc{dc}")
                dst3 = t[:, :].rearrange("p (nt c) -> p nt c", c=512)
                step = NT // pieces
                for g in range(pieces):
                    eng.dma_start(dst3[:, g * step:(g + 1) * step],
                                  xT_r[dc, :, g * step:(g + 1) * step])
                xtc[dc] = t

            load_chunk(0, nc.sync, pieces=4)
            load_chunk(1, nc.sync)
            load_chunk(2, nc.sync)

            ar_emitted = [False] * len(CH_DT)

            def emit_ar(q):
                # gpsimd carries the collectives: a collective blocks its
                # issuing queue until completion, so nothing time-critical
                # may queue behind one.
                nc.gpsimd.collective_compute(
                    "AllReduce", mybir.AluOpType.add,
                    replica_groups=groups,
                    ins=[tmp_in[q].opt()], outs=[tmp_out[q].opt()])
                ar_emitted[q] = True

            for dt in range(DT):
                dc = dt // 4
                if dt % 4 == 0 and dc + 3 < DC:
                    load_chunk(dc + 3, nc.sync)
                mm = [ps.tile([128, 512], F32, tag="mm1", bufs=6,
                              name=f"mm1_{dt}_{_oc}") for _oc in range(OC)]
                doff = (dt % 4) * 128
                for ntt in range(NT):
                    for oc in range(OC):
                        nc.tensor.matmul(
                            mm[oc][:],
                            xtc[dc][:, ntt * 512 + doff:ntt * 512 + doff + 128],
                            psiT_sb[oc][:, ntt * 512:(ntt + 1) * 512],
                            start=(ntt == 0), stop=(ntt == NT - 1))
                q = max(i for i in range(len(CH_DT)) if CH_START[i] <= dt)
                dq = dt - CH_START[q]
                for oc in range(OC):
                    st = sb1.tile([128, 512], BF, tag="t1s", bufs=24,
                                  name=f"t1s{dt}_{oc}")
                    nc.vector.tensor_copy(st[:], mm[oc][:])
                    nc.scalar.dma_start(
                        tmp_in[q][dq * 128:(dq + 1) * 128,
                                  oc * 512:(oc + 1) * 512],
                        st[:])
                if dq == CH_DT[q] - 1:
                    emit_ar(q)

        # Deferred-load discipline: any DMA that waits on a post-mm1 event
        # (AllReduce 3/4 completion, or pass-A matmuls via x2b buffer
        # rotation) must be EMITTED after everything mm1's drain depends
        # on. Tile assigns HWDGE completion-sem lanes round-robin in
        # emission order, so a late-completing DMA emitted early poisons
        # the lane pre-waits of later mm1 stage-outs — which the sb1
        # pool-close barrier (and with it the vector queue, the first
        # out_part copies, and AR4's trigger) then waits on. That was the
        # v12 32us PE bubble at the mm1->mm2 switch.
        def load_tmp_q(q):
            p = CH_START[q] // KH
            col0 = (CH_START[q] - p * KH) * OL
            src = tmp_out[q][:, :].rearrange("(dq p) o -> p dq o", p=128)
            dst = tmp_sb[p][:, col0:col0 + CH_DT[q] * OL].rearrange(
                "p (dq o) -> p dq o", o=OL)
            nc.sync.dma_start(dst, src)

        def load_x2b(p, ncn):
            t = sbl.tile([128, KH * 512], BF, tag="x2b", bufs=2,
                         name=f"x2b{p}_{ncn}")
            nc.sync.dma_start(
                t[:, :].rearrange("p (dq c) -> p dq c", c=512),
                x_r[p, ncn])
            x2b[(p, ncn)] = t

        # Safe to emit now: AR chunks 0-2 complete ~50us before mm1 ends.
        load_x2b(0, 0)
        load_x2b(0, 1)
        load_tmp_q(0)
        load_tmp_q(1)
        load_tmp_q(2)

        # Emission points for the rest, keyed by (pass, ncn) just completed:
        # x2b rotation (bufs=2) frees a buffer two sets ahead of its
        # consumer, and tmp chunks 3/4 go right after pass A's ncn2 so the
        # sync FIFO reaches them long before pass B starts (~40us margin).
        deferred = {
            (0, 0): [lambda: load_x2b(0, 2)],
            (0, 1): [lambda: load_x2b(0, 3)],
            (0, 2): [lambda: load_tmp_q(3), lambda: load_tmp_q(4),
                     lambda: load_x2b(1, 0)],
            (0, 3): [lambda: load_x2b(1, 1)],
            (1, 0): [lambda: load_x2b(1, 2)],
            (1, 1): [lambda: load_x2b(1, 3)],
        }

        # ============ mm2 ============
        with tc.tile_pool(name="sb2", bufs=1) as sb2:
            out_part = [sb2.tile([128, NL], F32, tag="out_part", bufs=OT,
                                 name=f"out_part{ot}") for ot in range(OT)]
            for p in range(2):
                for ncn in range(NCH):
                    for ot in range(OT):
                        mmo = ps.tile([128, 512], F32, tag="mm2", bufs=2,
                                      name=f"mm2_{p}_{ncn}_{ot}")
                        for dq in range(KH):
                            nc.tensor.matmul(
                                mmo[:],
                                tmp_sb[p][:, dq * OL + ot * 128:
                                          dq * OL + (ot + 1) * 128],
                                x2b[(p, ncn)][:, dq * 512:(dq + 1) * 512],
                                start=(dq == 0), stop=(dq == KH - 1))
                        if p == 0:
                            nc.vector.tensor_copy(
                                out_part[ot][:, ncn * 512:(ncn + 1) * 512],
                                mmo[:])
                        else:
                            ost = sb2.tile([128, 512], F32, tag="ost", bufs=4,
                                           name=f"ost{ot}_{ncn}")
                            nc.vector.tensor_tensor(
                                ost[:], mmo[:],
                                out_part[ot][:, ncn * 512:(ncn + 1) * 512],
                                op=mybir.AluOpType.add)
                            # alternate stage-out queues so the final ncn
                            # group's drain is split across two engines
                            eng = nc.scalar if ot % 2 == 0 else nc.sync
                            eng.dma_start(
                                out_ext[ot * 128:(ot + 1) * 128,
                                        ncn * 512:(ncn + 1) * 512],
                                ost[:])
                    for fn in deferred.pop((p, ncn), []):
                        fn()
    nc.compile()
    return nc


def make_in_maps(x, Psi, n_cores=8, NL=2048, OL=1024):
    """Shard full f32 inputs for the 4x2 grid with host-side prep:
    center Psi with the global row-mean, slice, transpose, cast bf16."""
    import numpy as np
    import ml_dtypes
    bf16 = ml_dtypes.bfloat16

    Psi_c = (Psi.astype(np.float64)
             - Psi.mean(axis=1, dtype=np.float64, keepdims=True))
    in_maps = []
    for c in range(n_cores):
        i, j = c % 4, c // 4
        xs = x[:, i * NL:(i + 1) * NL].astype(np.float32)
        ps_ = Psi_c[j * OL:(j + 1) * OL, i * NL:(i + 1) * NL]
        in_maps.append({
            "x": np.ascontiguousarray(xs).astype(bf16),
            "xT": np.ascontiguousarray(xs.T).astype(bf16),
            "psiT": np.ascontiguousarray(ps_.T).astype(bf16),
        })
    return in_maps


# ---------------- harness-facing wrapper ----------------
import numpy as np

_NC_CACHE = {}

D_FULL, N_FULL, O_FULL = 4096, 8192, 2048
NL_, OL_ = 2048, 1024
N_CORES = 8
GROUPS = ((0, 1, 2, 3), (4, 5, 6, 7))


def _get_nc():
    if "nc" not in _NC_CACHE:
        _NC_CACHE["nc"] = build_srp_kernel(
            D=D_FULL, NL=NL_, OL=OL_, n_cores=N_CORES, groups=GROUPS)
    return _NC_CACHE["nc"]


def kernel(x, Psi):
    """out = (Psi - rowmean(Psi)) @ x.T @ x on 8 TRN2 NeuronCores."""
    from concourse.bass_utils import run_bass_kernel_spmd
    x = np.asarray(x, dtype=np.float32)
    Psi = np.asarray(Psi, dtype=np.float32)
    assert x.shape == (D_FULL, N_FULL) and Psi.shape == (O_FULL, N_FULL)
    nc = _get_nc()
    in_maps = make_in_maps(x, Psi, n_cores=N_CORES, NL=NL_, OL=OL_)
    res = run_bass_kernel_spmd(nc, in_maps, core_ids=list(range(N_CORES)))
    out = np.empty((O_FULL, N_FULL), dtype=np.float32)
    for c in range(N_CORES):
        i, j = c % 4, c // 4
        out[j * OL_:(j + 1) * OL_, i * NL_:(i + 1) * NL_] = res.results[c]["out"]
    return out


# revision 10
# speedup vs baseline: 1.0695x; 1.0695x over previous
"""SRP layer distributed Bass kernel for TRN2 (v13).

Math (full problem): out = Psi_c @ x.T @ x with Psi_c = Psi - rowmean(Psi).
  x [D, N] f32, Psi [O, N] f32, out [O, N] f32  (D=4096, N=8192, O=2048)

Distribution over 8 cores as a 4x2 grid: core c -> (i = c % 4: n-quarter,
j = c // 4: o-half). The host pre-centers Psi (global row-mean), pre-slices,
pre-transposes, and pre-casts to bf16, so the device does NOTHING but the
two GEMMs and the tmp AllReduce:

Per core (NL = N/4 = 2048, OL = O/2 = 1024):
  xT   [NL, D]  bf16  (x_i.T)        - mm1 stationary operand
  x    [D, NL]  bf16  (x_i)          - mm2 moving operand
  psiT [NL, OL] bf16  (Psi_c_ji.T)   - mm1 moving operand
  out  [OL, NL] f32

mm1: tmpT[d, o] = sum_n xT[n, d] * psiT[n, o]   (partial over local n)
     -> bf16 -> DRAM in 5 d-chunks (4,4,8,8,8 d-tiles), each AllReduce'd
     over the 4 cores of the same o-half as soon as it is ready (the small
     leading chunks launch the first collective early, absorbing the ~30us
     cross-core start skew; the chain overlaps mm1 + mm2 pass A).
mm2: out[o, n] = sum_d tmpT[d, o] * x[d, n], two kd-half passes so pass A
     (kd 0..15, AR chunks 0-2) runs while chunks 3-4 still AllReduce;
     pass B adds and streams the f32 result out.

Perf notes (hardware-measured):
- Sustained MM pitch is 263 ns (512 cyc at 1.95 GHz: the package SW
  throttler holds k=13/16 under full-PE load; 2.4 GHz only in bursts),
  so the PE floor here is 2048 x 263 = 539 us.
- v12 measured 601 us with a 32 us PE bubble at the mm1->mm2 switch:
  the sb1 pool-close barrier counts ALL prior DMA-queue completions, and
  the tmp_sb loads for AR chunks 3-4 (issued inside the sb1 scope) kept
  the barrier - and with it the vector queue, the first out_part copies,
  and AR4's trigger - hostage to AllReduce-3 (~317 us > mm1 end ~298 us).
  v13 issues every AR- or mm2-dependent load AFTER the sb1 close, so the
  barrier clears the moment mm1's own traffic drains (~298 us).
- Bulk loads are single multi-dim DMAs (one per xT chunk / psiT half
  piece / tmp chunk / x2b block) spread over scalar+gpsimd+vector+sync
  queues: a single queue sustains only ~190 GB/s, and dt0 needs 5 MiB
  in its first ~19 us, so the initial fill uses four queues in parallel.
- Matmul operands slice a few BIG consolidated SBUF tiles: per-matmul
  semaphore waits otherwise break LDWEIGHTS pipelining (+45 ns/MM).
- gpsimd carries the collectives (they block their issuing queue) plus
  one initial psiT piece; nothing time-critical queues behind an AR.
- t1s staging is 24 deep and mm1 holds 6 PSUM banks so mm1 coasts
  through the DMA blackout during each AllReduce's transfer phase.
"""

from contextlib import ExitStack

import concourse.bacc as bacc
import concourse.mybir as mybir
import concourse.tile as tile

F32 = mybir.dt.float32
BF = mybir.dt.bfloat16


def build_srp_kernel(
    D=4096,
    NL=2048,
    OL=1024,
    n_cores=8,
    groups=((0, 1, 2, 3), (4, 5, 6, 7)),
):
    DT = D // 128    # 32 d-tiles (tmpT partition tiles / mm2 contraction)
    NT = NL // 128   # 16 n-tiles (mm1 contraction)
    OC = OL // 512   # 2  o-chunks (mm1 free cols)
    NCH = NL // 512  # 4  n-chunks (mm2 free cols)
    OT = OL // 128   # 8  o-tiles (mm2 output partition tiles)
    DC = D // 512    # 8  xT d-chunks (streamed)
    # AllReduce chunk sizes in d-tiles: small leading chunks so the first
    # collective launches early (it absorbs the ~30us cross-core arrival
    # skew) and the chain covering pass A (dt 0..15) completes well before
    # mm1 ends; chunks 0..2 -> mm2 pass A, chunks 3..4 -> pass B.
    CH_DT = (4, 4, 8, 8, 8)
    CH_START = [sum(CH_DT[:i]) for i in range(len(CH_DT))]
    KH = DT // 2     # 16 kd per mm2 pass

    groups = [list(g) for g in groups]

    nc = bacc.Bacc("TRN2", target_bir_lowering=False, debug=False,
                   num_devices=n_cores)
    xT_ext = nc.dram_tensor("xT", [NL, D], BF, kind="ExternalInput")
    x_ext = nc.dram_tensor("x", [D, NL], BF, kind="ExternalInput")
    psiT_ext = nc.dram_tensor("psiT", [NL, OL], BF, kind="ExternalInput")
    out_ext = nc.dram_tensor("out", [OL, NL], F32, kind="ExternalOutput")

    # [p, nt, c] views: partition-first on both sides of every bulk DMA.
    psiT_r = psiT_ext.rearrange("(nt p) (oc c) -> oc p nt c", p=128, c=512)
    xT_r = xT_ext.rearrange("(nt p) (dc c) -> dc p nt c", p=128, c=512)
    x_r = x_ext.rearrange("(ph dq p) (ncn c) -> ph ncn p dq c",
                          dq=KH, p=128, c=512)

    with ExitStack() as stack:
        tc = stack.enter_context(tile.TileContext(nc))
        dram = stack.enter_context(tc.tile_pool(name="dram", bufs=1, space="DRAM"))
        ps = stack.enter_context(tc.tile_pool(name="ps", bufs=1, space="PSUM"))
        sbl = stack.enter_context(tc.tile_pool(name="sbl", bufs=1))

        tmp_in = [dram.tile([CH_DT[q] * 128, OL], BF, tag=f"tmp_in{q}", bufs=1,
                            name=f"tmp_in{q}") for q in range(len(CH_DT))]
        tmp_out = [dram.tile([CH_DT[q] * 128, OL], BF, tag=f"tmp_out{q}", bufs=1,
                             name=f"tmp_out{q}") for q in range(len(CH_DT))]

        tmp_sb = [sbl.tile([128, KH * OL], BF, tag="tmp_sb", bufs=2,
                           name=f"tmp_sb{p}") for p in range(2)]
        x2b = {}

        # ============ mm1 ============
        with tc.tile_pool(name="sb1", bufs=1) as sb1:
            # psiT split into two o-halves, each one big tile [128, NT*512]
            # (block nt at cols nt*512..). Four 4-nt pieces per half so the
            # first MMs start as soon as piece 0 lands; oc0 on scalar,
            # oc1 on gpsimd (idle until AR0, ~70us in).
            psiT_sb = [sb1.tile([128, NT * 512], BF, tag=f"psiT{oc}", bufs=1,
                                name=f"psiT{oc}") for oc in range(OC)]
            for oc in range(OC):
                eng = nc.scalar if oc == 0 else nc.gpsimd
                dst3 = psiT_sb[oc][:, :].rearrange("p (nt c) -> p nt c", c=512)
                for g in range(4):
                    eng.dma_start(dst3[:, 4 * g:4 * (g + 1)],
                                  psiT_r[oc, :, 4 * g:4 * (g + 1)])

            # xT chunks: one big tile per 512-d-col chunk [128, NT*512],
            # rotating through 3 buffers, all on sync (DVE cannot issue
            # DMAs); chunk 0 in 4 pieces so MM0 starts at piece 0.
            xtc = {}

            def load_chunk(dc, eng, pieces=1):
                t = sb1.tile([128, NT * 512], BF, tag="xTc", bufs=3,
                             name=f"xTc{dc}")
                dst3 = t[:, :].rearrange("p (nt c) -> p nt c", c=512)
                step = NT // pieces
                for g in range(pieces):
                    eng.dma_start(dst3[:, g * step:(g + 1) * step],
                                  xT_r[dc, :, g * step:(g + 1) * step])
                xtc[dc] = t

            load_chunk(0, nc.sync, pieces=4)
            load_chunk(1, nc.sync)
            load_chunk(2, nc.sync)

            ar_emitted = [False] * len(CH_DT)

            def emit_ar(q):
                # gpsimd carries the collectives: a collective blocks its
                # issuing queue until completion, so nothing time-critical
                # may queue behind one.
                nc.gpsimd.collective_compute(
                    "AllReduce", mybir.AluOpType.add,
                    replica_groups=groups,
                    ins=[tmp_in[q].opt()], outs=[tmp_out[q].opt()])
                ar_emitted[q] = True

            def load_tmp_q(q):
                # tmp_sb loads ride the gpsimd SWDGE queue: its completion
                # sem lanes are disjoint from the HWDGE lanes, so these
                # AR-gated DMAs can never poison the lane pre-waits of mm1
                # stage-outs / AR triggers / the sb1 pool-close barrier
                # (the v12-v14 30-50us PE bubble at the mm1->mm2 switch),
                # and the queue's collective-blocking FIFO already
                # sequences them behind the AllReduce that writes them.
                p = CH_START[q] // KH
                col0 = (CH_START[q] - p * KH) * OL
                src = tmp_out[q][:, :].rearrange("(dq p) o -> p dq o", p=128)
                dst = tmp_sb[p][:, col0:col0 + CH_DT[q] * OL].rearrange(
                    "p (dq o) -> p dq o", o=OL)
                nc.gpsimd.dma_start(dst, src)

            for dt in range(DT):
                dc = dt // 4
                if dt % 4 == 0 and dc + 3 < DC:
                    load_chunk(dc + 3, nc.sync)
                mm = [ps.tile([128, 512], F32, tag="mm1", bufs=6,
                              name=f"mm1_{dt}_{_oc}") for _oc in range(OC)]
                doff = (dt % 4) * 128
                for ntt in range(NT):
                    for oc in range(OC):
                        nc.tensor.matmul(
                            mm[oc][:],
                            xtc[dc][:, ntt * 512 + doff:ntt * 512 + doff + 128],
                            psiT_sb[oc][:, ntt * 512:(ntt + 1) * 512],
                            start=(ntt == 0), stop=(ntt == NT - 1))
                q = max(i for i in range(len(CH_DT)) if CH_START[i] <= dt)
                dq = dt - CH_START[q]
                for oc in range(OC):
                    st = sb1.tile([128, 512], BF, tag="t1s", bufs=24,
                                  name=f"t1s{dt}_{oc}")
                    nc.vector.tensor_copy(st[:], mm[oc][:])
                    nc.scalar.dma_start(
                        tmp_in[q][dq * 128:(dq + 1) * 128,
                                  oc * 512:(oc + 1) * 512],
                        st[:])
                if dq == CH_DT[q] - 1:
                    emit_ar(q)
                    if q <= 2:
                        # pass-A chunk: its tmp_sb load slots between this
                        # AR's trigger and the next in the gpsimd FIFO, so
                        # it lands ~10us after its AR completes (mm1 still
                        # has >=40us to run at that point).
                        load_tmp_q(q)

        # Pass-B tmp chunks: emitted after the sb1 close so the close
        # barrier's lane snapshot excludes them; in the gpsimd FIFO they
        # sit behind AR4's trigger and land ~20us after AR4 completes,
        # still >30us before pass B consumes them.
        load_tmp_q(3)
        load_tmp_q(4)

        def load_x2b(p, ncn):
            t = sbl.tile([128, KH * 512], BF, tag="x2b", bufs=2,
                         name=f"x2b{p}_{ncn}")
            nc.sync.dma_start(
                t[:, :].rearrange("p (dq c) -> p dq c", c=512),
                x_r[p, ncn])
            x2b[(p, ncn)] = t

        # With x2b bufs=2, (p0,n2) reuses (p0,n0)'s buffer, so its DMA
        # waits on pass-A ncn0 matmuls (the scheduler models that dep and
        # places it after them — its lane slot stays clean); every x2b set
        # lands >25us before its consuming ncn iteration.
        for pncn in [(0, 0), (0, 1), (0, 2), (0, 3), (1, 0), (1, 1),
                     (1, 2), (1, 3)]:
            load_x2b(*pncn)

        # ============ mm2 ============
        with tc.tile_pool(name="sb2", bufs=1) as sb2:
            out_part = [sb2.tile([128, NL], F32, tag="out_part", bufs=OT,
                                 name=f"out_part{ot}") for ot in range(OT)]
            for p in range(2):
                for ncn in range(NCH):
                    for ot in range(OT):
                        mmo = ps.tile([128, 512], F32, tag="mm2", bufs=2,
                                      name=f"mm2_{p}_{ncn}_{ot}")
                        for dq in range(KH):
                            nc.tensor.matmul(
                                mmo[:],
                                tmp_sb[p][:, dq * OL + ot * 128:
                                          dq * OL + (ot + 1) * 128],
                                x2b[(p, ncn)][:, dq * 512:(dq + 1) * 512],
                                start=(dq == 0), stop=(dq == KH - 1))
                        if p == 0:
                            nc.vector.tensor_copy(
                                out_part[ot][:, ncn * 512:(ncn + 1) * 512],
                                mmo[:])
                        else:
                            ost = sb2.tile([128, 512], F32, tag="ost", bufs=4,
                                           name=f"ost{ot}_{ncn}")
                            nc.vector.tensor_tensor(
                                ost[:], mmo[:],
                                out_part[ot][:, ncn * 512:(ncn + 1) * 512],
                                op=mybir.AluOpType.add)
                            # alternate stage-out queues so the final ncn
                            # group's drain is split across two engines
                            eng = nc.scalar if ot % 2 == 0 else nc.sync
                            eng.dma_start(
                                out_ext[ot * 128:(ot + 1) * 128,
                                        ncn * 512:(ncn + 1) * 512],
                                ost[:])
    nc.compile()
    return nc


def make_in_maps(x, Psi, n_cores=8, NL=2048, OL=1024):
    """Shard full f32 inputs for the 4x2 grid with host-side prep:
    center Psi with the global row-mean, slice, transpose, cast bf16."""
    import numpy as np
    import ml_dtypes
    bf16 = ml_dtypes.bfloat16

    Psi_c = (Psi.astype(np.float64)
             - Psi.mean(axis=1, dtype=np.float64, keepdims=True))
    in_maps = []
    for c in range(n_cores):
        i, j = c % 4, c // 4
        xs = x[:, i * NL:(i + 1) * NL].astype(np.float32)
        ps_ = Psi_c[j * OL:(j + 1) * OL, i * NL:(i + 1) * NL]
        in_maps.append({
            "x": np.ascontiguousarray(xs).astype(bf16),
            "xT": np.ascontiguousarray(xs.T).astype(bf16),
            "psiT": np.ascontiguousarray(ps_.T).astype(bf16),
        })
    return in_maps


# ---------------- harness-facing wrapper ----------------
import numpy as np

_NC_CACHE = {}

D_FULL, N_FULL, O_FULL = 4096, 8192, 2048
NL_, OL_ = 2048, 1024
N_CORES = 8
GROUPS = ((0, 1, 2, 3), (4, 5, 6, 7))


def _get_nc():
    if "nc" not in _NC_CACHE:
        _NC_CACHE["nc"] = build_srp_kernel(
            D=D_FULL, NL=NL_, OL=OL_, n_cores=N_CORES, groups=GROUPS)
    return _NC_CACHE["nc"]


def kernel(x, Psi):
    """out = (Psi - rowmean(Psi)) @ x.T @ x on 8 TRN2 NeuronCores."""
    from concourse.bass_utils import run_bass_kernel_spmd
    x = np.asarray(x, dtype=np.float32)
    Psi = np.asarray(Psi, dtype=np.float32)
    assert x.shape == (D_FULL, N_FULL) and Psi.shape == (O_FULL, N_FULL)
    nc = _get_nc()
    in_maps = make_in_maps(x, Psi, n_cores=N_CORES, NL=NL_, OL=OL_)
    res = run_bass_kernel_spmd(nc, in_maps, core_ids=list(range(N_CORES)))
    out = np.empty((O_FULL, N_FULL), dtype=np.float32)
    for c in range(N_CORES):
        i, j = c % 4, c // 4
        out[j * OL_:(j + 1) * OL_, i * NL_:(i + 1) * NL_] = res.results[c]["out"]
    return out
